# revision 2
# baseline (speedup 1.0000x reference)
"""Trainium2 Bass kernel for ChannelMaxPool top-k masking.

Reference computation:
  x: (B=32, C=512, H=128, W=128) f32
  scores[b,c] = max |x[b,c,:,:]|
  top-128 channels by score (descending, jax.lax.top_k tie order)
  w[b,k] = exp(s_k - m) / sum_selected exp(s_j - m)
  y[b,k,:,:] = x[b, idx_k, :, :] * w[b,k]

Sharding: pure data-parallel, batch split across 8 NeuronCores
(4 samples per core), no communication.

Zero-gather scheme (vs. the 192 MiB/core load+gather+store baseline):
the score pass streams x once in f32; while DVE absmax-reduces exact
scores, the Scalar engine down-converts every tile into a bf16 copy of
the whole sample kept in SBUF (16 MiB -> 128 KiB/partition).  After the
exact f32 top-k, rank and weight are transposed from rank-order to
channel-order with one-hot fp16 matmuls on the otherwise idle tensor
engine (exactly one 1.0 per output => exact rank arithmetic).  Selected
cache rows are scaled in place and written straight from SBUF to y with
an indirect DMA whose per-row destination index is the rank; unselected
channels carry an out-of-bounds sentinel, which the DMA silently skips
(no traffic).  y is bf16 (~0.4% worst-case element error, well inside
the 2e-2 gate) and upconverted on the host.

Traffic/core: 128 MiB load + 16 MiB bf16 store = 144 MiB
(~405 us HBM roofline at 356 GB/s, vs 632 us measured baseline).

The bf16 cache uses 5 rotating group slots (slot = (4b+g) mod 5): the
slot a sample converts group g into was last read by the scatter one
full sample earlier, so converts never stall on this sample's epilogue.
"""

import numpy as np

B, C, H, W = 32, 512, 128, 128
S = H * W
K = 128
N_CORES = 8
BL = B // N_CORES

S_TILE = 4096
CSLOTS = 5
NEG_INF = -1e30
BIG = 1 << 20


def _build_nc():
    import concourse.bass as bass
    import concourse.mybir as mybir
    from concourse import bacc
    from concourse.tile import TileContext

    f32 = mybir.dt.float32
    f16 = mybir.dt.float16
    bf16 = mybir.dt.bfloat16
    u32 = mybir.dt.uint32
    i32 = mybir.dt.int32

    CCH = C // 128  # 4 channel groups of 128
    NT = S // S_TILE  # tiles per channel group

    nc = bacc.Bacc()
    x = nc.dram_tensor("x", [BL, C, S], f32, kind="ExternalInput")
    y = nc.dram_tensor("y", [BL, K, S], bf16, kind="ExternalOutput")

    y_rows = y[:].rearrange("b k s -> (b k) s")

    with TileContext(nc) as tc:
        with (
            tc.tile_pool(name="load", bufs=2) as load_pool,
            tc.tile_pool(name="cache", bufs=1) as cache_pool,
            tc.tile_pool(name="psum", bufs=4, space="PSUM") as psum_pool,
            tc.tile_pool(name="small", bufs=2) as small,
            tc.tile_pool(name="single", bufs=1) as single,
        ):
            # ---- constants ----
            # per-partition row 0..C-1 (f32 exact for these values)
            iota_c = single.tile([128, C], f32, tag="iota_c")
            nc.gpsimd.iota(
                iota_c[:],
                pattern=[[1, C]],
                base=0,
                channel_multiplier=0,
                allow_small_or_imprecise_dtypes=True,
            )
            # rank+1 per partition, fp16 (<=128, exact)
            ranks_i = single.tile([K, 1], i32, tag="ranks_i")
            nc.gpsimd.iota(ranks_i[:], pattern=[[1, 1]], base=1, channel_multiplier=1)
            ranks_h = single.tile([K, 1], f16, tag="ranks_h")
            nc.vector.tensor_copy(ranks_h[:], ranks_i[:])

            # whole-sample bf16 cache, 5 rotating group slots
            cache = cache_pool.tile([128, CSLOTS * S], bf16, tag="cache")

            for b in range(BL):
                # ---- pass 1: stream tiles; exact absmax scores on DVE,
                #      bf16 conversion into the cache on Scalar ----
                FINE = 4  # sub-splits of the very last tile (topk starts earlier)
                last_sample = b == BL - 1
                n_par = CCH * NT + (FINE - 1 if last_sample else 0)
                partials = small.tile([128, CCH * NT + FINE - 1], f32, tag="partials")
                for ci in range(CCH):
                    slot = (b * CCH + ci) % CSLOTS
                    for t in range(NT):
                        last_tile = last_sample and ci == CCH - 1 and t == NT - 1
                        sub = FINE if last_tile else 1
                        sw = S_TILE // sub
                        for u in range(sub):
                            tile_in = load_pool.tile([128, S_TILE], f32, tag="ld")
                            s0 = t * S_TILE + u * sw
                            nc.sync.dma_start(
                                out=tile_in[:, :sw],
                                in_=x[b, ci * 128 : (ci + 1) * 128, s0 : s0 + sw],
                            )
                            col = ci * NT + t + u
                            nc.vector.tensor_reduce(
                                out=partials[:, col : col + 1],
                                in_=tile_in[:, :sw],
                                axis=mybir.AxisListType.X,
                                op=mybir.AluOpType.max,
                                apply_absolute_value=True,
                            )
                            nc.scalar.activation(
                                out=cache[:, slot * S + s0 : slot * S + s0 + sw],
                                in_=tile_in[:, :sw],
                                func=mybir.ActivationFunctionType.Copy,
                                bias=0.0,
                                scale=1.0,
                            )
                scores_col = small.tile([128, CCH], f32, tag="scores_col")
                if not last_sample:
                    nc.vector.tensor_reduce(
                        out=scores_col[:],
                        in_=partials[:, : CCH * NT].rearrange("p (g t) -> p g t", t=NT),
                        axis=mybir.AxisListType.X,
                        op=mybir.AluOpType.max,
                    )
                else:
                    nc.vector.tensor_reduce(
                        out=scores_col[:, : CCH - 1],
                        in_=partials[:, : (CCH - 1) * NT].rearrange(
                            "p (g t) -> p g t", t=NT
                        ),
                        axis=mybir.AxisListType.X,
                        op=mybir.AluOpType.max,
                    )
                    nc.vector.tensor_reduce(
                        out=scores_col[:, CCH - 1 : CCH],
                        in_=partials[:, None, (CCH - 1) * NT : n_par],
                        axis=mybir.AxisListType.X,
                        op=mybir.AluOpType.max,
                    )
                # ---- transpose scores to one row via SBUF->SBUF DMAs ----
                scores_row = small.tile([1, C], f32, tag="scores_row")
                for ci in range(CCH):
                    nc.sync.dma_start(
                        out=scores_row[:, ci * 128 : (ci + 1) * 128],
                        in_=scores_col[:, ci : ci + 1],
                    )
                # ---- top-K via repeated top-8 extraction (descending),
                #      consuming scores_row in place ----
                topk_vals = small.tile([1, K], f32, tag="topk_vals")
                topk_idx = small.tile([1, K], u32, tag="topk_idx")
                for i in range(K // 8):
                    sl = slice(i * 8, (i + 1) * 8)
                    nc.vector.max(out=topk_vals[:, sl], in_=scores_row[:])
                    nc.vector.max_index(
                        out=topk_idx[:, sl],
                        in_max=topk_vals[:, sl],
                        in_values=scores_row[:],
                    )
                    if i < K // 8 - 1:
                        nc.vector.match_replace(
                            out=scores_row[:],
                            in_to_replace=topk_vals[:, sl],
                            in_values=scores_row[:],
                            imm_value=NEG_INF,
                        )
                # ---- one-hot (rank x channel) for the rank/weight transpose ----
                idx_col_u = small.tile([K, 1], u32, tag="idx_col_u")
                nc.sync.dma_start(out=idx_col_u[:], in_=topk_idx[:])
                idx_col_f = small.tile([K, 1], f32, tag="idx_col_f")
                nc.vector.tensor_copy(idx_col_f[:], idx_col_u[:])
                onehot = small.tile([K, C], f16, tag="onehot")
                nc.vector.tensor_tensor(
                    out=onehot[:],
                    in0=idx_col_f[:].to_broadcast([K, C]),
                    in1=iota_c[:],
                    op=mybir.AluOpType.is_equal,
                )
                # ---- weights from the exact f32 top-k values ----
                negm = small.tile([1, 1], f32, tag="negm")
                nc.scalar.mul(out=negm[:], in_=topk_vals[:, 0:1], mul=-1.0)
                e = small.tile([1, K], f32, tag="e")
                nc.scalar.activation(
                    out=e[:],
                    in_=topk_vals[:],
                    func=mybir.ActivationFunctionType.Exp,
                    bias=negm[:, 0:1],
                    scale=1.0,
                )
                ssum = small.tile([1, 1], f32, tag="ssum")
                nc.vector.reduce_sum(out=ssum[:], in_=e[:], axis=mybir.AxisListType.X)
                sinv = small.tile([1, 1], f32, tag="sinv")
                nc.vector.reciprocal(out=sinv[:], in_=ssum[:])
                w_row = small.tile([1, K], f32, tag="w_row")
                nc.vector.tensor_scalar_mul(w_row[:], e[:], sinv[:, 0:1])
                w_col = small.tile([K, 1], f32, tag="w_col")
                nc.sync.dma_start(out=w_col[:], in_=w_row[:])
                # moving operand: [rank+1, w] per rank (fp16: rank exact,
                # w to ~0.05%)
                vals_h = small.tile([K, 2], f16, tag="vals_h")
                nc.vector.tensor_copy(vals_h[:, 0:1], ranks_h[:])
                nc.vector.tensor_copy(vals_h[:, 1:2], w_col[:])
                # ---- per-channel (rank+1, w) via one-hot matmuls ----
                rw = small.tile([128, CCH, 2], f32, tag="rw")
                for g in range(CCH):
                    ps = psum_pool.tile([128, 2], f32, tag="ps")
                    nc.tensor.matmul(
                        ps[:],
                        onehot[:, g * 128 : (g + 1) * 128],
                        vals_h[:],
                    )
                    nc.vector.tensor_copy(rw[:, g, :], ps[:])
                # dest row index: rank + b*K if selected else OOB sentinel
                # v = rank+1 (0 if unselected):
                #   off = v + (b*K - 1 + BIG) - BIG*min(v,1)
                mb4 = small.tile([128, CCH], f32, tag="mb4")
                nc.vector.tensor_scalar(
                    out=mb4[:],
                    in0=rw[:, :, 0],
                    scalar1=1.0,
                    scalar2=float(BIG),
                    op0=mybir.AluOpType.min,
                    op1=mybir.AluOpType.mult,
                )
                off_f = small.tile([128, CCH], f32, tag="off_f")
                nc.vector.scalar_tensor_tensor(
                    out=off_f[:],
                    in0=rw[:, :, 0],
                    scalar=float(b * K - 1 + BIG),
                    in1=mb4[:],
                    op0=mybir.AluOpType.add,
                    op1=mybir.AluOpType.subtract,
                )
                off_i = small.tile([128, CCH], i32, tag="off_i")
                nc.vector.tensor_copy(off_i[:], off_f[:])
                # ---- scale cached rows in place, scatter straight to y;
                #      rows with OOB rank are silently skipped ----
                for g in range(CCH):
                    slot = (b * CCH + g) % CSLOTS
                    cache_g = cache[:, slot * S : (slot + 1) * S]
                    if g % 2 == 0:
                        nc.vector.tensor_scalar_mul(cache_g, cache_g, rw[:, g, 1:2])
                    else:
                        nc.scalar.activation(
                            out=cache_g,
                            in_=cache_g,
                            func=mybir.ActivationFunctionType.Copy,
                            bias=0.0,
                            scale=rw[:, g, 1:2],
                        )
                    nc.gpsimd.indirect_dma_start(
                        out=y_rows,
                        out_offset=bass.IndirectOffsetOnAxis(
                            ap=off_i[:, g : g + 1], axis=0
                        ),
                        in_=cache_g,
                        in_offset=None,
                        bounds_check=BL * K - 1,
                        oob_is_err=False,
                    )
    if not nc.is_finalized():
        nc.finalize()
    return nc


_NC_CACHE = None


def _get_nc():
    global _NC_CACHE
    if _NC_CACHE is None:
        _NC_CACHE = _build_nc()
    return _NC_CACHE


def _run(x, trace=False):
    from concourse.bass_utils import run_bass_kernel_spmd

    nc = _get_nc()
    xr = np.ascontiguousarray(x, dtype=np.float32).reshape(N_CORES, BL, C, S)
    in_maps = [{"x": xr[c]} for c in range(N_CORES)]
    res = run_bass_kernel_spmd(nc, in_maps, list(range(N_CORES)), trace=trace)
    out = np.empty((B, K, H, W), dtype=np.float32)
    for c in range(N_CORES):
        out[c * BL : (c + 1) * BL] = (
            np.asarray(res.results[c]["y"]).astype(np.float32).reshape(BL, K, H, W)
        )
    return out, res


def kernel(x):
    out, _ = _run(x, trace=False)
    return out
